# revision 18
# baseline (speedup 1.0000x reference)
"""Trainium2 Bass kernel for a locally-connected Conv2d (nn.Conv2dLocal).

Problem shapes (hardcoded):
  x      [B=64, Cin=32, H=32, W=32]  fp32
  weight [OH=30, OW=30, Cout=64, Cin=32, KH=3, KW=3] fp32 (per-location weights)
  bias   [Cout=64, OH=30, OW=30] fp32
  out    [B=64, Cout=64, OH=30, OW=30] fp32

Strategy: shard the 30 output rows across 8 cores (4 padded rows per core).
The kernel is HBM-DMA bound and the per-location weights dominate the
traffic, so the weights ship as int8 (scale ws = 127/absmax, quantized on
host).  HWDGE DMA cannot cast and SWDGE cast-DMA descriptor generation is
too slow for this volume, so the int8 chunks land in SBUF staging tiles via
plain HWDGE transfers and are expanded to bf16 on-chip, each chunk split
between DVE tensor_copy (~0.58 ns/el-lane) and ACT copy (~0.94) so both
engines chase the stream in parallel.  int8 casts to bf16 exactly, bf16
products of int values are exact, and fp32 PSUM accumulation of these sums
is exact, so the device adds no error beyond the host-side quantization
(~5e-3 measured vs the 2e-2 budget; x stays bf16).  The bias is added on
the host after the 1/ws rescale, which keeps every transfer at 96
partitions — HWDGE fans 96-row transfers across all 16 SDMA engines, while
a 97th row would serialize the whole transfer onto one engine (measured).

Per output row h, an SBUF tile xh[h] holds three input image rows as
[(kh, ci) -> 96 partitions, (img col c, batch b) free].  For each image
column c the stationary operand xh[:, c] is shared by up to three (wl, kw)
weight taps (wl + kw == c); the per-location weights stream as the moving
operand from per-h chunk tiles, packed on the host so each step's taps are
contiguous.  Accumulation is in PSUM: one bank holds 8 output locations
(64 cols each); per-element has_written bits make the first tap overwrite
and later taps accumulate.

DMA rings: sync (HWDGE) carries the x mains and int8 weight chunks in
consumption order; scalar carries the dep-gated per-h output ships so they
overlap the load stream on a separate queue.  Drains alternate DVE / ACT.
"""

import os
import sys

import numpy as np

for _p in ("/opt/trn_rl_repo", "/root/.axon_site/_ro/trn_rl_repo"):
    if os.path.isdir(_p) and _p not in sys.path:
        sys.path.insert(0, _p)

import concourse.bass as bass  # noqa: E402
import concourse.tile as tile  # noqa: E402
from concourse import bacc, mybir  # noqa: E402
from concourse.bass_utils import run_bass_kernel_spmd  # noqa: E402

import ml_dtypes  # noqa: E402

F32 = mybir.dt.float32
BF16 = mybir.dt.bfloat16
I8 = mybir.dt.int8
NP_BF16 = ml_dtypes.bfloat16

# problem constants
B, CI, H, W = 64, 32, 32, 32
CO = 64
KH = KW = 3
OH = OW = 30
NCORES = 8
RPC = 4  # padded output rows per core (8 * 4 = 32 >= 30)
OHP = NCORES * RPC  # 32
HPAD = OHP + KH - 1  # 34 padded input rows
K96 = KH * CI  # 96 contraction rows
XROWS = (RPC + 2) * CI  # 192 x rows per core

# (c, j, wl) pair enumeration: j descending within each c so that psum slots
# (wl % 8) ascend within a segment, matching the moving-operand column order.
PAIRS = []
for _c in range(W):
    for _j in (2, 1, 0):
        _wl = _c - _j
        if 0 <= _wl < OW:
            PAIRS.append((_c, _j, _wl))
NPAIRS = len(PAIRS)  # 90
WPANEL = NPAIRS * CO  # 5760 elements per h panel

# Weight chunking: the dep tracker coalesces adjacent writes within a tile,
# so chase granularity requires SEPARATE tiles.  Chunk boundaries must fall
# on c boundaries (pair index where a new c starts) so no matmul segment
# straddles tiles.  h0 gets a small first chunk so the PE starts early; h3
# uses thirds so the expand+compute tail after the last weight byte stays
# short.
CHUNKS_BY_H = [
    [(0, 45), (45, 90)],
    [(0, 45), (45, 90)],
    [(0, 45), (45, 90)],
    [(0, 30), (30, 60), (60, 78), (78, 90)],
]

# pair index -> (chunk index, chunk lo) per h
PAIR_CHUNK = {}
for _h in range(RPC):
    for _k, (_lo, _hi) in enumerate(CHUNKS_BY_H[_h]):
        for _i in range(_lo, _hi):
            PAIR_CHUNK[(_h, _i)] = (_k, _lo)


def _build_segments():
    """Per-c matmul segments: lists of consecutive (pair_idx, (c, j, wl)).

    A segment's taps land in one PSUM bank with ascending slots.  start=True
    is used only for the matmul that is the first write of a whole bank
    (slot 0's j==0 tap) — it clears the bank's has_written bits; hardware's
    per-element has_written bits then make each slot's first tap overwrite
    and later taps accumulate.
    """
    segments = {c: [] for c in range(W)}
    for c in range(W):
        pairs = [(i, PAIRS[i]) for i in range(NPAIRS) if PAIRS[i][0] == c]
        seg = []
        for i, (cc, j, wl) in pairs:
            if seg and (seg[-1][1][2] // 8) != (wl // 8):
                segments[c].append(seg)
                seg = []
            seg.append((i, (cc, j, wl)))
        if seg:
            segments[c].append(seg)
    return segments


# drain bank `beta` right after processing column c == last write for the bank
DRAIN_AFTER_C = {}
for beta in range(4):
    last_wl = min(8 * beta + 7, OW - 1)
    DRAIN_AFTER_C.setdefault(last_wl + 2, []).append(beta)

_CACHED = {}


def _build_nc():
    """Build the single-core SPMD Bass program (identical on all 8 cores)."""
    from contextlib import ExitStack

    segments = _build_segments()
    nc = bacc.Bacc("TRN2", target_bir_lowering=False, debug=False,
                   num_devices=NCORES)
    x_d = nc.dram_tensor("x", [XROWS, W * B], BF16,
                         kind="ExternalInput").ap()
    w_d = nc.dram_tensor("w", [K96, RPC * WPANEL], I8,
                         kind="ExternalInput").ap()
    o_d = nc.dram_tensor("o", [B, RPC * OW * CO], BF16,
                         kind="ExternalOutput").ap()

    with tile.TileContext(nc) as tc, ExitStack() as ctx:
        xpool = ctx.enter_context(tc.tile_pool(name="xh", bufs=1))
        spool = ctx.enter_context(tc.tile_pool(name="w8", bufs=1))
        wpool = ctx.enter_context(tc.tile_pool(name="wt", bufs=1))
        opool = ctx.enter_context(tc.tile_pool(name="ob", bufs=1))
        ppool = ctx.enter_context(
            tc.tile_pool(name="ps", bufs=8, space=bass.MemorySpace.PSUM))

        xh = [xpool.tile([K96, W * B], BF16, name=f"xh{h}", tag=f"xh{h}")
              for h in range(RPC)]
        w8s, wts = {}, {}
        for h in range(RPC):
            for k, (lo, hi) in enumerate(CHUNKS_BY_H[h]):
                w8s[(h, k)] = spool.tile([K96, (hi - lo) * CO], I8,
                                         name=f"w8_{h}_{k}", tag=f"w8_{h}_{k}")
                wts[(h, k)] = wpool.tile([K96, (hi - lo) * CO], BF16,
                                         name=f"wt{h}_{k}", tag=f"wt{h}_{k}")
        out_sb = opool.tile([B, RPC * OW * CO], BF16, name="ob", tag="ob")

        # all loads ride the sync HWDGE ring (96-row transfers fan across
        # all 16 SDMA engines; the queue is FIFO so arrival order is issue
        # order).  The x blocks need no expansion, so all but x0 ride LATE —
        # weight chunks land earlier and the expansion engines clear their
        # backlog before the stream ends instead of ~4us after.  Each x
        # block still arrives just before its h's matmuls need it.  Ships
        # ride scalar so they overlap on a separate queue.
        def xdma(h):
            nc.sync.dma_start(xh[h][0:K96, :], x_d[CI * h:CI * h + K96, :])

        def wdma(h, k):
            lo, hi = CHUNKS_BY_H[h][k]
            nc.sync.dma_start(
                w8s[(h, k)],
                w_d[0:K96, h * WPANEL + lo * CO:h * WPANEL + hi * CO])

        wdma(0, 0)
        wdma(0, 1)
        xdma(0)
        wdma(1, 0)
        wdma(1, 1)
        xdma(1)
        wdma(2, 0)
        wdma(2, 1)
        xdma(2)
        wdma(3, 0)
        wdma(3, 1)
        xdma(3)
        wdma(3, 2)
        wdma(3, 3)

        # int8 -> bf16 expansions: each chunk split DVE/ACT by columns
        # (DVE ~1.63x faster at SBUF casts) so both engines chase the stream
        for h in range(RPC):
            for k, (lo, hi) in enumerate(CHUNKS_BY_H[h]):
                n = (hi - lo) * CO
                nd = n if n <= 512 else (int(n * 0.62) // 64) * 64
                src, dst = w8s[(h, k)], wts[(h, k)]
                nc.vector.tensor_copy(dst[:, 0:nd], src[:, 0:nd])
                if nd < n:
                    nc.scalar.copy(dst[:, nd:n], src[:, nd:n])

        for h in range(RPC):
            xt = xh[h]
            psums = {}
            for c in range(W):
                lhs = xt[:, c * B:(c + 1) * B]  # [96, 64] stationary
                for seg in segments[c]:
                    i0 = seg[0][0]
                    npair = len(seg)
                    wl0 = seg[0][1][2]
                    beta = wl0 // 8
                    slot0 = wl0 % 8
                    # first write of the whole bank: slot0's j==0 tap (it is
                    # always a single-pair segment since its bank-mates
                    # belong to the previous bank)
                    start = (npair == 1 and seg[0][1][1] == 0 and slot0 == 0)
                    stop = (npair == 1 and seg[0][1][1] == 2
                            and (wl0 % 8 == 7 or wl0 == OW - 1))
                    k, clo = PAIR_CHUNK[(h, i0)]
                    wtile = wts[(h, k)]
                    rhs = wtile[:, (i0 - clo) * CO:(i0 - clo + npair) * CO]
                    if beta not in psums:
                        psums[beta] = ppool.tile([B, 512], F32,
                                                 name=f"ps_h{h}_b{beta}",
                                                 tag="ps")
                    out_ap = psums[beta][:, slot0 * CO:(slot0 + npair) * CO]
                    nc.tensor.matmul(out_ap, lhs, rhs, start=start, stop=stop,
                                     skip_group_check=True)
                for beta in DRAIN_AFTER_C.get(c, []):
                    nslot = min(8, OW - 8 * beta)
                    pt = psums.pop(beta)
                    dst = out_sb[:, h * OW * CO + beta * 8 * CO:
                                 h * OW * CO + (beta * 8 + nslot) * CO]
                    # alternate drain engines so neither becomes the pacer
                    if beta % 2 == 0:
                        nc.vector.tensor_copy(dst, pt[:, :nslot * CO])
                    else:
                        nc.scalar.copy(dst, pt[:, :nslot * CO])
                    # ship per bank: each ship gates on exactly one drain,
                    # so the final ship (h3 bank3) is small and starts the
                    # moment the last drain lands
                    lo = h * OW * CO + beta * 8 * CO
                    hi = h * OW * CO + (beta * 8 + nslot) * CO
                    nc.scalar.dma_start(o_d[:, lo:hi], out_sb[:, lo:hi])
    nc.compile()
    return nc


def _prep_inputs(x, weight, bias):
    """Host-side shard + relayout + int8 weight quantization.

    Returns (in_maps, ws): per-core input dicts and the weight scale the
    output must be divided by (bias is added on the host in _assemble).
    """
    x = np.ascontiguousarray(np.asarray(x, dtype=np.float32))
    weight = np.ascontiguousarray(np.asarray(weight, dtype=np.float32))

    x_pad = np.zeros((B, CI, HPAD, W), np.float32)
    x_pad[:, :, :H, :] = x
    # [r, ci, w, b]
    x_t = np.ascontiguousarray(x_pad.transpose(2, 1, 3, 0)).astype(NP_BF16)

    w_pad = np.zeros((OHP, OW, CO, CI, KH, KW), np.float32)
    w_pad[:OH] = weight
    # [oh, kh, ci, wl, kw, o] -> [oh, 96, wl, kw, o]
    w4 = w_pad.transpose(0, 4, 3, 1, 5, 2).reshape(OHP, K96, OW, KW, CO)

    wl_list = np.array([wl for (c, j, wl) in PAIRS])
    j_list = np.array([j for (c, j, wl) in PAIRS])
    # [oh, 96, 90, 64]: contraction row p = (kh*32 + ci)
    w2 = w4[:, :, wl_list, j_list, :]

    # int8 quantization with one global scale
    ws = 127.0 / np.abs(weight).max()
    w2_q = np.clip(np.rint(w2 * ws), -127, 127).astype(np.int8)

    in_maps = []
    for core in range(NCORES):
        r0 = RPC * core
        xc = np.ascontiguousarray(
            x_t[r0:r0 + RPC + 2].reshape(XROWS, W * B))
        # [96, (h, pair, co)]
        wc = np.ascontiguousarray(
            w2_q[r0:r0 + RPC].transpose(1, 0, 2, 3).reshape(
                K96, RPC * WPANEL))
        in_maps.append({"x": xc, "w": wc})
    return in_maps, ws


def _assemble(results, ws, bias):
    inv = np.float32(1.0 / ws)
    out = np.empty((B, CO, OH, OW), np.float32)
    for core in range(NCORES):
        oc = (results[core]["o"].astype(np.float32) * inv).reshape(
            B, RPC, OW, CO).transpose(0, 3, 1, 2)
        r0 = RPC * core
        r1 = min(r0 + RPC, OH)
        if r1 > r0:
            out[:, :, r0:r1, :] = oc[:, :, :r1 - r0, :]
    return out + np.asarray(bias, np.float32)[None]


def run(x, weight, bias, trace=False, **trace_kwargs):
    """Build (cached), run on 8 cores, return (output, BassKernelResults)."""
    if "nc" not in _CACHED:
        _CACHED["nc"] = _build_nc()
    nc = _CACHED["nc"]
    in_maps, ws = _prep_inputs(x, weight, bias)
    res = run_bass_kernel_spmd(nc, in_maps, list(range(NCORES)),
                               trace=trace, **trace_kwargs)
    return _assemble(res.results, ws, bias), res


def kernel(x, weight, bias):
    out, _ = run(x, weight, bias)
    return out


# revision 20
# speedup vs baseline: 1.1540x; 1.1540x over previous
"""Trainium2 Bass kernel for a locally-connected Conv2d (nn.Conv2dLocal).

Problem shapes (hardcoded):
  x      [B=64, Cin=32, H=32, W=32]  fp32
  weight [OH=30, OW=30, Cout=64, Cin=32, KH=3, KW=3] fp32 (per-location weights)
  bias   [Cout=64, OH=30, OW=30] fp32
  out    [B=64, Cout=64, OH=30, OW=30] fp32

Strategy: shard the 30 output rows across 8 cores (4 padded rows per core).
The kernel is HBM-DMA bound and the per-location weights dominate the
traffic, so the weights ship as int8 (scale ws = 127/absmax, quantized on
host).  HWDGE DMA cannot cast and SWDGE cast-DMA descriptor generation is
too slow for this volume, so the int8 chunks land in SBUF staging tiles via
plain HWDGE transfers and are expanded to bf16 on-chip, each chunk split
between DVE tensor_copy (~0.58 ns/el-lane) and ACT copy (~0.94) so both
engines chase the stream in parallel.  int8 casts to bf16 exactly, bf16
products of int values are exact, and fp32 PSUM accumulation of these sums
is exact, so the device adds no error beyond the host-side quantization
(~5e-3 measured vs the 2e-2 budget; x stays bf16).  The bias is added on
the host after the 1/ws rescale, which keeps every transfer at 96
partitions — HWDGE fans 96-row transfers across all 16 SDMA engines, while
a 97th row would serialize the whole transfer onto one engine (measured).

Per output row h, an SBUF tile xh[h] holds three input image rows as
[(kh, ci) -> 96 partitions, (img col c, batch b) free].  For each image
column c the stationary operand xh[:, c] is shared by up to three (wl, kw)
weight taps (wl + kw == c); the per-location weights stream as the moving
operand from per-h chunk tiles, packed on the host so each step's taps are
contiguous.  Accumulation is in PSUM: one bank holds 8 output locations
(64 cols each); per-element has_written bits make the first tap overwrite
and later taps accumulate.

DMA rings: sync (HWDGE) carries the x mains and int8 weight chunks in
consumption order; scalar carries the dep-gated per-h output ships so they
overlap the load stream on a separate queue.  Drains alternate DVE / ACT.
"""

import os
import sys

import numpy as np

for _p in ("/opt/trn_rl_repo", "/root/.axon_site/_ro/trn_rl_repo"):
    if os.path.isdir(_p) and _p not in sys.path:
        sys.path.insert(0, _p)

import concourse.bass as bass  # noqa: E402
import concourse.tile as tile  # noqa: E402
from concourse import bacc, mybir  # noqa: E402
from concourse.bass_utils import run_bass_kernel_spmd  # noqa: E402

import ml_dtypes  # noqa: E402

F32 = mybir.dt.float32
BF16 = mybir.dt.bfloat16
I8 = mybir.dt.int8
NP_BF16 = ml_dtypes.bfloat16

# problem constants
B, CI, H, W = 64, 32, 32, 32
CO = 64
KH = KW = 3
OH = OW = 30
NCORES = 8
RPC = 4  # padded output rows per core (8 * 4 = 32 >= 30)
OHP = NCORES * RPC  # 32
HPAD = OHP + KH - 1  # 34 padded input rows
K96 = KH * CI  # 96 contraction rows
XROWS = (RPC + 2) * CI  # 192 x rows per core

# (c, j, wl) pair enumeration: j descending within each c so that psum slots
# (wl % 8) ascend within a segment, matching the moving-operand column order.
PAIRS = []
for _c in range(W):
    for _j in (2, 1, 0):
        _wl = _c - _j
        if 0 <= _wl < OW:
            PAIRS.append((_c, _j, _wl))
NPAIRS = len(PAIRS)  # 90
WPANEL = NPAIRS * CO  # 5760 elements per h panel

# Weight chunking: the dep tracker coalesces adjacent writes within a tile,
# so chase granularity requires SEPARATE tiles.  Chunk boundaries must fall
# on c boundaries (pair index where a new c starts) so no matmul segment
# straddles tiles.  h0 gets a small first chunk so the PE starts early; h3
# uses thirds so the expand+compute tail after the last weight byte stays
# short.
CHUNKS_BY_H = [
    [(0, 45), (45, 90)],
    [(0, 45), (45, 90)],
    [(0, 45), (45, 90)],
    [(0, 30), (30, 60), (60, 78), (78, 90)],
]

# pair index -> (chunk index, chunk lo) per h
PAIR_CHUNK = {}
for _h in range(RPC):
    for _k, (_lo, _hi) in enumerate(CHUNKS_BY_H[_h]):
        for _i in range(_lo, _hi):
            PAIR_CHUNK[(_h, _i)] = (_k, _lo)


def _build_segments():
    """Per-c matmul segments: lists of consecutive (pair_idx, (c, j, wl)).

    A segment's taps land in one PSUM bank with ascending slots.  start=True
    is used only for the matmul that is the first write of a whole bank
    (slot 0's j==0 tap) — it clears the bank's has_written bits; hardware's
    per-element has_written bits then make each slot's first tap overwrite
    and later taps accumulate.
    """
    segments = {c: [] for c in range(W)}
    for c in range(W):
        pairs = [(i, PAIRS[i]) for i in range(NPAIRS) if PAIRS[i][0] == c]
        seg = []
        for i, (cc, j, wl) in pairs:
            if seg and (seg[-1][1][2] // 8) != (wl // 8):
                segments[c].append(seg)
                seg = []
            seg.append((i, (cc, j, wl)))
        if seg:
            segments[c].append(seg)
    return segments


# drain bank `beta` right after processing column c == last write for the bank
DRAIN_AFTER_C = {}
for beta in range(4):
    last_wl = min(8 * beta + 7, OW - 1)
    DRAIN_AFTER_C.setdefault(last_wl + 2, []).append(beta)

_CACHED = {}


def _build_nc():
    """Build the single-core SPMD Bass program (identical on all 8 cores)."""
    from contextlib import ExitStack

    segments = _build_segments()
    nc = bacc.Bacc("TRN2", target_bir_lowering=False, debug=False,
                   num_devices=NCORES)
    x_d = nc.dram_tensor("x", [XROWS, W * B], BF16,
                         kind="ExternalInput").ap()
    w_d = nc.dram_tensor("w", [K96, RPC * WPANEL], I8,
                         kind="ExternalInput").ap()
    o_d = nc.dram_tensor("o", [B, RPC * OW * CO], BF16,
                         kind="ExternalOutput").ap()

    with tile.TileContext(nc) as tc, ExitStack() as ctx:
        xpool = ctx.enter_context(tc.tile_pool(name="xh", bufs=1))
        spool = ctx.enter_context(tc.tile_pool(name="w8", bufs=1))
        wpool = ctx.enter_context(tc.tile_pool(name="wt", bufs=1))
        opool = ctx.enter_context(tc.tile_pool(name="ob", bufs=1))
        ppool = ctx.enter_context(
            tc.tile_pool(name="ps", bufs=8, space=bass.MemorySpace.PSUM))

        xh = [xpool.tile([K96, W * B], BF16, name=f"xh{h}", tag=f"xh{h}")
              for h in range(RPC)]
        w8s, wts = {}, {}
        for h in range(RPC):
            for k, (lo, hi) in enumerate(CHUNKS_BY_H[h]):
                w8s[(h, k)] = spool.tile([K96, (hi - lo) * CO], I8,
                                         name=f"w8_{h}_{k}", tag=f"w8_{h}_{k}")
                wts[(h, k)] = wpool.tile([K96, (hi - lo) * CO], BF16,
                                         name=f"wt{h}_{k}", tag=f"wt{h}_{k}")
        out_sb = opool.tile([B, RPC * OW * CO], BF16, name="ob", tag="ob")

        # all loads ride the sync HWDGE ring (96-row transfers fan across
        # all 16 SDMA engines; the queue is FIFO so arrival order is issue
        # order).  The x blocks need no expansion, so all but x0 ride LATE —
        # weight chunks land earlier and the expansion engines clear their
        # backlog before the stream ends instead of ~4us after.  Each x
        # block still arrives just before its h's matmuls need it.  Ships
        # ride scalar so they overlap on a separate queue.
        def xdma(h):
            nc.sync.dma_start(xh[h][0:K96, :], x_d[CI * h:CI * h + K96, :])

        def wdma(h, k):
            lo, hi = CHUNKS_BY_H[h][k]
            nc.sync.dma_start(
                w8s[(h, k)],
                w_d[0:K96, h * WPANEL + lo * CO:h * WPANEL + hi * CO])

        wdma(0, 0)
        wdma(0, 1)
        xdma(0)
        wdma(1, 0)
        wdma(1, 1)
        xdma(1)
        wdma(2, 0)
        wdma(2, 1)
        xdma(2)
        wdma(3, 0)
        wdma(3, 1)
        xdma(3)
        wdma(3, 2)
        wdma(3, 3)

        # int8 -> bf16 expansions: each chunk split DVE/ACT by columns
        # (DVE ~1.63x faster at SBUF casts) so both engines chase the stream
        for h in range(RPC):
            for k, (lo, hi) in enumerate(CHUNKS_BY_H[h]):
                n = (hi - lo) * CO
                nd = n if n <= 512 else (int(n * 0.62) // 64) * 64
                src, dst = w8s[(h, k)], wts[(h, k)]
                nc.vector.tensor_copy(dst[:, 0:nd], src[:, 0:nd])
                if nd < n:
                    nc.scalar.copy(dst[:, nd:n], src[:, nd:n])

        for h in range(RPC):
            xt = xh[h]
            psums = {}
            for c in range(W):
                lhs = xt[:, c * B:(c + 1) * B]  # [96, 64] stationary
                for seg in segments[c]:
                    i0 = seg[0][0]
                    npair = len(seg)
                    wl0 = seg[0][1][2]
                    beta = wl0 // 8
                    slot0 = wl0 % 8
                    # first write of the whole bank: slot0's j==0 tap (it is
                    # always a single-pair segment since its bank-mates
                    # belong to the previous bank)
                    start = (npair == 1 and seg[0][1][1] == 0 and slot0 == 0)
                    stop = (npair == 1 and seg[0][1][1] == 2
                            and (wl0 % 8 == 7 or wl0 == OW - 1))
                    k, clo = PAIR_CHUNK[(h, i0)]
                    wtile = wts[(h, k)]
                    rhs = wtile[:, (i0 - clo) * CO:(i0 - clo + npair) * CO]
                    if beta not in psums:
                        psums[beta] = ppool.tile([B, 512], F32,
                                                 name=f"ps_h{h}_b{beta}",
                                                 tag="ps")
                    out_ap = psums[beta][:, slot0 * CO:(slot0 + npair) * CO]
                    nc.tensor.matmul(out_ap, lhs, rhs, start=start, stop=stop,
                                     skip_group_check=True)
                for beta in DRAIN_AFTER_C.get(c, []):
                    nslot = min(8, OW - 8 * beta)
                    pt = psums.pop(beta)
                    dst = out_sb[:, h * OW * CO + beta * 8 * CO:
                                 h * OW * CO + (beta * 8 + nslot) * CO]
                    # alternate drain engines so neither becomes the pacer
                    if beta % 2 == 0:
                        nc.vector.tensor_copy(dst, pt[:, :nslot * CO])
                    else:
                        nc.scalar.copy(dst, pt[:, :nslot * CO])
                    if beta % 2 == 1:  # banks (beta-1, beta) drained -> ship
                        # ships ride SYNC: a gated DIRECT2D stalls its
                        # issuing sequencer, and sync is idle after load
                        # emission, while scalar would block ACT's own
                        # drains/expansions behind the stall (16 per-bank
                        # ships on scalar measured 3.2us worse for this
                        # reason)
                        lo = h * OW * CO + (beta - 1) * 8 * CO
                        hi = h * OW * CO + (beta * 8 + nslot) * CO
                        nc.sync.dma_start(o_d[:, lo:hi], out_sb[:, lo:hi])
    nc.compile()
    return nc


def _prep_inputs(x, weight, bias):
    """Host-side shard + relayout + int8 weight quantization.

    Returns (in_maps, ws): per-core input dicts and the weight scale the
    output must be divided by (bias is added on the host in _assemble).
    """
    x = np.ascontiguousarray(np.asarray(x, dtype=np.float32))
    weight = np.ascontiguousarray(np.asarray(weight, dtype=np.float32))

    x_pad = np.zeros((B, CI, HPAD, W), np.float32)
    x_pad[:, :, :H, :] = x
    # [r, ci, w, b]
    x_t = np.ascontiguousarray(x_pad.transpose(2, 1, 3, 0)).astype(NP_BF16)

    w_pad = np.zeros((OHP, OW, CO, CI, KH, KW), np.float32)
    w_pad[:OH] = weight
    # [oh, kh, ci, wl, kw, o] -> [oh, 96, wl, kw, o]
    w4 = w_pad.transpose(0, 4, 3, 1, 5, 2).reshape(OHP, K96, OW, KW, CO)

    wl_list = np.array([wl for (c, j, wl) in PAIRS])
    j_list = np.array([j for (c, j, wl) in PAIRS])
    # [oh, 96, 90, 64]: contraction row p = (kh*32 + ci)
    w2 = w4[:, :, wl_list, j_list, :]

    # int8 quantization with one global scale
    ws = 127.0 / np.abs(weight).max()
    w2_q = np.clip(np.rint(w2 * ws), -127, 127).astype(np.int8)

    in_maps = []
    for core in range(NCORES):
        r0 = RPC * core
        xc = np.ascontiguousarray(
            x_t[r0:r0 + RPC + 2].reshape(XROWS, W * B))
        # [96, (h, pair, co)]
        wc = np.ascontiguousarray(
            w2_q[r0:r0 + RPC].transpose(1, 0, 2, 3).reshape(
                K96, RPC * WPANEL))
        in_maps.append({"x": xc, "w": wc})
    return in_maps, ws


def _assemble(results, ws, bias):
    inv = np.float32(1.0 / ws)
    out = np.empty((B, CO, OH, OW), np.float32)
    for core in range(NCORES):
        oc = (results[core]["o"].astype(np.float32) * inv).reshape(
            B, RPC, OW, CO).transpose(0, 3, 1, 2)
        r0 = RPC * core
        r1 = min(r0 + RPC, OH)
        if r1 > r0:
            out[:, :, r0:r1, :] = oc[:, :, :r1 - r0, :]
    return out + np.asarray(bias, np.float32)[None]


def run(x, weight, bias, trace=False, **trace_kwargs):
    """Build (cached), run on 8 cores, return (output, BassKernelResults)."""
    if "nc" not in _CACHED:
        _CACHED["nc"] = _build_nc()
    nc = _CACHED["nc"]
    in_maps, ws = _prep_inputs(x, weight, bias)
    res = run_bass_kernel_spmd(nc, in_maps, list(range(NCORES)),
                               trace=trace, **trace_kwargs)
    return _assemble(res.results, ws, bias), res


def kernel(x, weight, bias):
    out, _ = run(x, weight, bias)
    return out
